# revision 21
# baseline (speedup 1.0000x reference)
import sys

sys.path.insert(0, "/opt/trn_rl_repo")

import numpy as np
import ml_dtypes

import concourse.bass as bass
import concourse.bacc as bacc
import concourse.mybir as mybir
from concourse.tile import TileContext
from concourse.bass_utils import run_bass_kernel_spmd

P = 9
C = 64            # out channels
CIN = 32          # x in channels
CFE = 64          # y in channels
NCORES = 8
CPC = C // NCORES  # channels per core

D1, H1, W1 = 36, 72, 72
HW1 = H1 * W1                 # 5184
L1 = (D1 // P) * (HW1 // P)   # 4*576 = 2304
D2, H2, W2 = 18, 36, 36
HW2 = H2 * W2                 # 1296
L2 = (D2 // P) * (HW2 // P)   # 2*144 = 288

S = np.float64(1.0) / (np.float64(L2) + np.float64(1e-5))   # 1/nz
INV_S = float(1.0 / S)                                      # 288.00001
BF16 = ml_dtypes.bfloat16

# per-channel compute tiles: (col offset, width, A-step engine); matmuls run
# in 512-col sub-chunks into a shared [81, width] PSUM tile, then one
# leaky-relu instruction covers the whole tile
CTILES = [(0, 896, "vector"), (896, 1024, "scalar"), (1920, 384, "gpsimd")]
PREFETCH = 3
# device returns m = lrelu(corr); host applies the residual gating
# out = (m + 1/S) * S * zu during the fold it already performs
HOST_RESIDUAL = True
# output DMA split points (column ranges)
OUT_HALVES = [(0, 1024), (1024, 1280)]
GI_SPLIT = 4  # gi arrives as GI_SPLIT independent tiles (2 channels each)


def _unfold9(img):
    # (C, H, W) -> (C, 81, L)
    c, h, w = img.shape
    x = img.reshape(c, h // P, P, w // P, P)
    return np.ascontiguousarray(
        x.transpose(0, 2, 4, 1, 3).reshape(c, P * P, (h // P) * (w // P))
    )


def _fold9(blocks, h, w):
    # (C, 81, L) -> (C, H, W)
    c = blocks.shape[0]
    x = blocks.reshape(c, P, P, h // P, w // P)
    return x.transpose(0, 3, 1, 4, 2).reshape(c, h, w)


def _avgpool3d_k3s2p1(v):
    # (C, D, H, W) -> (C, D//2, H//2, W//2), count_include_pad=False
    c, d, h, w = v.shape
    pad = np.zeros((c, d + 2, h + 2, w + 2), np.float32)
    pad[:, 1:-1, 1:-1, 1:-1] = v
    one = np.zeros((d + 2, h + 2, w + 2), np.float32)
    one[1:-1, 1:-1, 1:-1] = 1.0
    s = np.zeros((c, d // 2, h // 2, w // 2), np.float32)
    cnt = np.zeros((d // 2, h // 2, w // 2), np.float32)
    for dz in range(3):
        for dy in range(3):
            for dx in range(3):
                s += pad[:, dz : dz + d : 2, dy : dy + h : 2, dx : dx + w : 2]
                cnt += one[dz : dz + d : 2, dy : dy + h : 2, dx : dx + w : 2]
    return s / cnt[None]


_NC_CACHE = {}


def _build_nc():
    if "nc" in _NC_CACHE:
        return _NC_CACHE["nc"]
    f32 = mybir.dt.float32
    bf16 = mybir.dt.bfloat16
    LR = mybir.ActivationFunctionType.Lrelu
    nc = bacc.Bacc(None, target_bir_lowering=False)
    # gi: per channel 486 cols = [uyT 3x81 | uxdT 3x81], all CPC channels packed
    gi = nc.dram_tensor("gi", [96, CPC * 486], bf16, kind="ExternalInput")
    ux = nc.dram_tensor("ux", [CPC, P * P, L1], bf16, kind="ExternalInput")
    out = nc.dram_tensor("out", [CPC, P * P, L1], bf16, kind="ExternalOutput")

    gchan = CPC // GI_SPLIT  # channels per gi chunk
    with TileContext(nc) as tc:
        with (
            tc.tile_pool(name="cst", bufs=1) as cp,
            tc.tile_pool(name="small", bufs=3) as sp,
            tc.tile_pool(name="inb", bufs=PREFETCH + 1) as bp,
            tc.tile_pool(name="ob", bufs=3) as op_,
            tc.tile_pool(name="psg", bufs=2, space="PSUM") as pg,
            tc.tile_pool(name="psc", bufs=3, space="PSUM") as pp,
        ):
            ux_ts = {}
            gi_ts = {}

            def fetch(c):
                ux_t = bp.tile([81, L1], bf16, tag="ux")
                nc.sync.dma_start(out=ux_t[:, :], in_=ux[c])
                ux_ts[c] = ux_t

            def fetch_gi(g):
                gi_t = cp.tile([96, gchan * 486], bf16, tag=f"gi{g}")
                nc.sync.dma_start(
                    out=gi_t[:, :],
                    in_=gi[:, g * gchan * 486 : (g + 1) * gchan * 486],
                )
                gi_ts[g] = gi_t

            fetch_gi(0)
            fetch(0)
            for g in range(1, GI_SPLIT):
                fetch_gi(g)
            for c in range(1, PREFETCH):
                fetch(c)

            gt_sbs = {}

            def gram(c):
                gi_t = gi_ts[c // gchan]
                gt_ps = pg.tile([81, 81], f32, tag="gt")
                base = (c % gchan) * 486
                for j in range(3):
                    nc.tensor.matmul(
                        gt_ps[:, :],
                        lhsT=gi_t[:, base + j * 81 : base + (j + 1) * 81],
                        rhs=gi_t[:, base + 243 + j * 81 : base + 243 + (j + 1) * 81],
                        start=(j == 0),
                        stop=(j == 2),
                    )
                gt_sb = sp.tile([81, 81], bf16, tag="gts")
                nc.gpsimd.tensor_copy(gt_sb[:, :], gt_ps[:, :])
                gt_sbs[c] = gt_sb

            gram(0)
            for c in range(CPC):
                ux_t = ux_ts.pop(c)
                if c + 1 < CPC:
                    gram(c + 1)
                gt_sb = gt_sbs.pop(c)

                out_t = op_.tile([81, L1], bf16, tag="o")
                for lo, w, a_eng in CTILES:
                    cor_ps = pp.tile([81, 1024], f32, tag="cor")
                    for sub in range(0, w, 512):
                        sw = min(512, w - sub)
                        nc.tensor.matmul(
                            cor_ps[:, sub : sub + sw],
                            lhsT=gt_sb[:, :],
                            rhs=ux_t[:, lo + sub : lo + sub + sw],
                            start=True,
                            stop=True,
                        )
                    # m = lrelu(corr) straight into the output tile
                    eng = getattr(nc, a_eng)
                    if a_eng == "scalar":
                        eng.activation(
                            out_t[:, lo : lo + w], cor_ps[:, :w], LR, alpha=0.2
                        )
                    else:
                        eng.scalar_tensor_tensor(
                            out_t[:, lo : lo + w],
                            cor_ps[:, :w],
                            0.2,
                            cor_ps[:, :w],
                            op0=mybir.AluOpType.mult,
                            op1=mybir.AluOpType.max,
                        )
                if c + PREFETCH < CPC:
                    fetch(c + PREFETCH)
                for lo, w in OUT_HALVES:
                    nc.sync.dma_start(
                        out=out[c, :, lo : lo + w], in_=out_t[:, lo : lo + w]
                    )
    nc.finalize()
    _NC_CACHE["nc"] = nc
    return nc


def kernel(x, y, z, w_img, b_img, w_fea, b_fea):
    x = np.asarray(x, np.float32)
    y = np.asarray(y, np.float32)
    z = np.asarray(z, np.float32)
    w_img = np.asarray(w_img, np.float32)
    b_img = np.asarray(b_img, np.float32)
    w_fea = np.asarray(w_fea, np.float32)
    b_fea = np.asarray(b_fea, np.float32)

    # host prep: pointwise projections (tiny) + layout permutes (zero-FLOP)
    x2 = x.reshape(CIN, D1, HW1)
    xq = (w_img @ x2.reshape(CIN, -1)).reshape(C, D1, HW1) + b_img[:, None, None]
    ux = _unfold9(xq)                                   # (C, 81, L1)

    y2 = y.reshape(CFE, D2, HW2)
    yk = (w_fea @ y2.reshape(CFE, -1)).reshape(C, D2, HW2) + b_fea[:, None, None]
    uyT = np.ascontiguousarray(
        _unfold9(yk).transpose(0, 2, 1).reshape(C, 3, 96, 81).transpose(0, 2, 1, 3)
    ).reshape(C, 96, 243)                               # (C, 96, (chunk,81))

    z4 = z.reshape(C, D1, H1, W1)
    xd = _avgpool3d_k3s2p1(z4).reshape(C, D2, HW2)
    uxdT = np.ascontiguousarray(
        _unfold9(xd).transpose(0, 2, 1).reshape(C, 3, 96, 81).transpose(0, 2, 1, 3)
    ).reshape(C, 96, 243)

    gi = np.concatenate([uyT, uxdT], axis=2)            # (C, 96, 486)
    ux16 = ux.astype(BF16)
    gi16 = gi.astype(BF16)

    nc = _build_nc()
    in_maps = []
    for k in range(NCORES):
        s = slice(k * CPC, (k + 1) * CPC)
        im = {
            "gi": np.ascontiguousarray(
                gi16[s].transpose(1, 0, 2).reshape(96, CPC * 486)
            ),
            "ux": np.ascontiguousarray(ux16[s]),
        }
        if not HOST_RESIDUAL:
            im["zs"] = np.ascontiguousarray(
                (np.float32(S) * _unfold9(z.reshape(C, D1, HW1))[s]).astype(BF16)
            )
        in_maps.append(im)
    res = run_bass_kernel_spmd(nc, in_maps, list(range(NCORES))).results
    outu = np.concatenate(
        [np.asarray(r["out"]).astype(np.float32) for r in res], axis=0
    )  # (C,81,L1)
    if HOST_RESIDUAL:
        # out = (m + 1/S) * S * zu  ==  (lrelu(S*corr) + 1) * zu
        zu = _unfold9(z.reshape(C, D1, HW1))
        outu = (outu + np.float32(INV_S)) * (np.float32(S) * zu)
    out = _fold9(outu, D1, HW1)
    return out.reshape(1, C, D1, H1, W1).astype(np.float32)


# revision 22
# speedup vs baseline: 1.0356x; 1.0356x over previous
import sys

sys.path.insert(0, "/opt/trn_rl_repo")

import numpy as np
import ml_dtypes

import concourse.bass as bass
import concourse.bacc as bacc
import concourse.mybir as mybir
from concourse.tile import TileContext
from concourse.bass_utils import run_bass_kernel_spmd

P = 9
C = 64            # out channels
CIN = 32          # x in channels
CFE = 64          # y in channels
NCORES = 8
CPC = C // NCORES  # channels per core

D1, H1, W1 = 36, 72, 72
HW1 = H1 * W1                 # 5184
L1 = (D1 // P) * (HW1 // P)   # 4*576 = 2304
D2, H2, W2 = 18, 36, 36
HW2 = H2 * W2                 # 1296
L2 = (D2 // P) * (HW2 // P)   # 2*144 = 288

S = np.float64(1.0) / (np.float64(L2) + np.float64(1e-5))   # 1/nz
INV_S = float(1.0 / S)                                      # 288.00001
BF16 = ml_dtypes.bfloat16

# per-channel compute tiles: (col offset, width, A-step engine); matmuls run
# in 512-col sub-chunks into a shared [81, width] PSUM tile, then one
# leaky-relu instruction covers the whole tile
CTILES = [(0, 1024, "vector"), (1024, 1024, "scalar"), (2048, 256, "gpsimd")]
PREFETCH = 3
# device returns m = lrelu(corr); host applies the residual gating
# out = (m + 1/S) * S * zu during the fold it already performs
HOST_RESIDUAL = True
# output DMA split points (column ranges)
OUT_HALVES = [(0, 1024), (1024, 1280)]
GI_SPLIT = 4  # gi arrives as GI_SPLIT independent tiles (2 channels each)


def _unfold9(img):
    # (C, H, W) -> (C, 81, L)
    c, h, w = img.shape
    x = img.reshape(c, h // P, P, w // P, P)
    return np.ascontiguousarray(
        x.transpose(0, 2, 4, 1, 3).reshape(c, P * P, (h // P) * (w // P))
    )


def _fold9(blocks, h, w):
    # (C, 81, L) -> (C, H, W)
    c = blocks.shape[0]
    x = blocks.reshape(c, P, P, h // P, w // P)
    return x.transpose(0, 3, 1, 4, 2).reshape(c, h, w)


def _avgpool3d_k3s2p1(v):
    # (C, D, H, W) -> (C, D//2, H//2, W//2), count_include_pad=False
    c, d, h, w = v.shape
    pad = np.zeros((c, d + 2, h + 2, w + 2), np.float32)
    pad[:, 1:-1, 1:-1, 1:-1] = v
    one = np.zeros((d + 2, h + 2, w + 2), np.float32)
    one[1:-1, 1:-1, 1:-1] = 1.0
    s = np.zeros((c, d // 2, h // 2, w // 2), np.float32)
    cnt = np.zeros((d // 2, h // 2, w // 2), np.float32)
    for dz in range(3):
        for dy in range(3):
            for dx in range(3):
                s += pad[:, dz : dz + d : 2, dy : dy + h : 2, dx : dx + w : 2]
                cnt += one[dz : dz + d : 2, dy : dy + h : 2, dx : dx + w : 2]
    return s / cnt[None]


_NC_CACHE = {}


def _build_nc():
    if "nc" in _NC_CACHE:
        return _NC_CACHE["nc"]
    f32 = mybir.dt.float32
    bf16 = mybir.dt.bfloat16
    LR = mybir.ActivationFunctionType.Lrelu
    nc = bacc.Bacc(None, target_bir_lowering=False)
    # gi: per channel 486 cols = [uyT 3x81 | uxdT 3x81], all CPC channels packed
    gi = nc.dram_tensor("gi", [96, CPC * 486], bf16, kind="ExternalInput")
    ux = nc.dram_tensor("ux", [CPC, P * P, L1], bf16, kind="ExternalInput")
    out = nc.dram_tensor("out", [CPC, P * P, L1], bf16, kind="ExternalOutput")

    gchan = CPC // GI_SPLIT  # channels per gi chunk
    with TileContext(nc) as tc:
        with (
            tc.tile_pool(name="cst", bufs=1) as cp,
            tc.tile_pool(name="small", bufs=3) as sp,
            tc.tile_pool(name="inb", bufs=PREFETCH + 1) as bp,
            tc.tile_pool(name="ob", bufs=3) as op_,
            tc.tile_pool(name="psg", bufs=2, space="PSUM") as pg,
            tc.tile_pool(name="psc", bufs=3, space="PSUM") as pp,
        ):
            ux_ts = {}
            gi_ts = {}

            def fetch(c):
                ux_t = bp.tile([81, L1], bf16, tag="ux")
                nc.sync.dma_start(out=ux_t[:, :], in_=ux[c])
                ux_ts[c] = ux_t

            def fetch_gi(g):
                gi_t = cp.tile([96, gchan * 486], bf16, tag=f"gi{g}")
                nc.sync.dma_start(
                    out=gi_t[:, :],
                    in_=gi[:, g * gchan * 486 : (g + 1) * gchan * 486],
                )
                gi_ts[g] = gi_t

            fetch_gi(0)
            fetch(0)
            for g in range(1, GI_SPLIT):
                fetch_gi(g)
            for c in range(1, PREFETCH):
                fetch(c)

            gt_sbs = {}

            def gram(c):
                gi_t = gi_ts[c // gchan]
                gt_ps = pg.tile([81, 81], f32, tag="gt")
                base = (c % gchan) * 486
                for j in range(3):
                    nc.tensor.matmul(
                        gt_ps[:, :],
                        lhsT=gi_t[:, base + j * 81 : base + (j + 1) * 81],
                        rhs=gi_t[:, base + 243 + j * 81 : base + 243 + (j + 1) * 81],
                        start=(j == 0),
                        stop=(j == 2),
                    )
                gt_sb = sp.tile([81, 81], bf16, tag="gts")
                nc.gpsimd.tensor_copy(gt_sb[:, :], gt_ps[:, :])
                gt_sbs[c] = gt_sb

            gram(0)
            for c in range(CPC):
                ux_t = ux_ts.pop(c)
                if c + 1 < CPC:
                    gram(c + 1)
                gt_sb = gt_sbs.pop(c)

                out_t = op_.tile([81, L1], bf16, tag="o")
                for lo, w, a_eng in CTILES:
                    cor_ps = pp.tile([81, 1024], f32, tag="cor")
                    for sub in range(0, w, 512):
                        sw = min(512, w - sub)
                        nc.tensor.matmul(
                            cor_ps[:, sub : sub + sw],
                            lhsT=gt_sb[:, :],
                            rhs=ux_t[:, lo + sub : lo + sub + sw],
                            start=True,
                            stop=True,
                        )
                    # m = lrelu(corr) straight into the output tile
                    eng = getattr(nc, a_eng)
                    if a_eng == "scalar":
                        eng.activation(
                            out_t[:, lo : lo + w], cor_ps[:, :w], LR, alpha=0.2
                        )
                    else:
                        eng.scalar_tensor_tensor(
                            out_t[:, lo : lo + w],
                            cor_ps[:, :w],
                            0.2,
                            cor_ps[:, :w],
                            op0=mybir.AluOpType.mult,
                            op1=mybir.AluOpType.max,
                        )
                if c + PREFETCH < CPC:
                    fetch(c + PREFETCH)
                for lo, w in OUT_HALVES:
                    nc.sync.dma_start(
                        out=out[c, :, lo : lo + w], in_=out_t[:, lo : lo + w]
                    )
    nc.finalize()
    _NC_CACHE["nc"] = nc
    return nc


def kernel(x, y, z, w_img, b_img, w_fea, b_fea):
    x = np.asarray(x, np.float32)
    y = np.asarray(y, np.float32)
    z = np.asarray(z, np.float32)
    w_img = np.asarray(w_img, np.float32)
    b_img = np.asarray(b_img, np.float32)
    w_fea = np.asarray(w_fea, np.float32)
    b_fea = np.asarray(b_fea, np.float32)

    # host prep: pointwise projections (tiny) + layout permutes (zero-FLOP)
    x2 = x.reshape(CIN, D1, HW1)
    xq = (w_img @ x2.reshape(CIN, -1)).reshape(C, D1, HW1) + b_img[:, None, None]
    ux = _unfold9(xq)                                   # (C, 81, L1)

    y2 = y.reshape(CFE, D2, HW2)
    yk = (w_fea @ y2.reshape(CFE, -1)).reshape(C, D2, HW2) + b_fea[:, None, None]
    uyT = np.ascontiguousarray(
        _unfold9(yk).transpose(0, 2, 1).reshape(C, 3, 96, 81).transpose(0, 2, 1, 3)
    ).reshape(C, 96, 243)                               # (C, 96, (chunk,81))

    z4 = z.reshape(C, D1, H1, W1)
    xd = _avgpool3d_k3s2p1(z4).reshape(C, D2, HW2)
    uxdT = np.ascontiguousarray(
        _unfold9(xd).transpose(0, 2, 1).reshape(C, 3, 96, 81).transpose(0, 2, 1, 3)
    ).reshape(C, 96, 243)

    gi = np.concatenate([uyT, uxdT], axis=2)            # (C, 96, 486)
    ux16 = ux.astype(BF16)
    gi16 = gi.astype(BF16)

    nc = _build_nc()
    in_maps = []
    for k in range(NCORES):
        s = slice(k * CPC, (k + 1) * CPC)
        im = {
            "gi": np.ascontiguousarray(
                gi16[s].transpose(1, 0, 2).reshape(96, CPC * 486)
            ),
            "ux": np.ascontiguousarray(ux16[s]),
        }
        if not HOST_RESIDUAL:
            im["zs"] = np.ascontiguousarray(
                (np.float32(S) * _unfold9(z.reshape(C, D1, HW1))[s]).astype(BF16)
            )
        in_maps.append(im)
    res = run_bass_kernel_spmd(nc, in_maps, list(range(NCORES))).results
    outu = np.concatenate(
        [np.asarray(r["out"]).astype(np.float32) for r in res], axis=0
    )  # (C,81,L1)
    if HOST_RESIDUAL:
        # out = (m + 1/S) * S * zu  ==  (lrelu(S*corr) + 1) * zu
        zu = _unfold9(z.reshape(C, D1, HW1))
        outu = (outu + np.float32(INV_S)) * (np.float32(S) * zu)
    out = _fold9(outu, D1, HW1)
    return out.reshape(1, C, D1, H1, W1).astype(np.float32)
